# revision 20
# baseline (speedup 1.0000x reference)
"""Self-contained Trainium2 Bass kernel for the bidirectional-LSTM decoder
(nn_Decoder): 2-direction LSTM scan over T=100 steps with a fixed input,
followed by a 32000-way vocab projection and log_softmax, on 8 NeuronCores.

Distribution (single fused SPMD launch):
  - Scan: direction-split. Cores 0-3 compute the forward LSTM for all 64
    batch rows, cores 4-7 the backward one. Each core receives a quarter of
    its direction's weights; full matrices are reassembled on-device with
    AllGather so host->device staging stays small.
  - Transpose-free scan layout: gates^T[4H, B] accumulated as
    sum_j Whh^T-tile[j, m].T @ h^T-chunk[j], state kept as [128, j, b].
    Gate rows are PERMUTED host-side to pair-major order
    (i,i,f,f,o,o,g,g per slice pair) so the per-step nonlinearity +
    cell update runs at slice-pair granularity pipelined against the
    matmuls of later gate chunks (sub-tile dependency tracking), instead
    of one serial chain after the full 4H x B gate tile.
  - LSTM outputs stay on-device: pair-AllGather gives every core both
    directions.
  - fc + log_softmax: vocab-split (4000 rows per core, fc_W resident in
    SBUF, bf16). SINGLE matmul pass: logits (+bias) are written once as
    bf16 both to SBUF stats (exp + accumulate -> softmax partial sums)
    and to internal DRAM. Partial exp-sums are AllReduced in 5 groups of
    10 token-tiles so the normalizer exchange and the epilogue overlap
    the remaining matmul work; the epilogue re-reads the bf16 logits,
    adds -logZ via an Identity activation with per-partition bias, and
    writes the final fp32 output. This replaces the baseline's full
    second recompute pass (~1.3 ms of PE time) with ~100 MB of
    overlapped DMA traffic.
  - All matmuls run in bf16 (fp32 accumulation in PSUM); cell state c
    stays fp32.
"""

import sys

if "/opt/trn_rl_repo" not in sys.path:
    sys.path.insert(0, "/opt/trn_rl_repo")

from contextlib import ExitStack

import numpy as np

import concourse.bass as bass
import concourse.tile as tile
from concourse import mybir
from concourse.bass_utils import run_bass_kernel_spmd

F32 = mybir.dt.float32
BF16 = mybir.dt.bfloat16
NP_BF16 = mybir.dt.np(BF16)
NP_W8 = mybir.dt.np(mybir.dt.float8e4)
B = 64
H = 1024
V = 32000
NCORES = 8

MAX_WAITS = 1

# Pair-major gate-chunk permutation: new chunk n (of 32) -> original chunk
# gate*8 + slice, with per-pair order [i_2p, i_2p+1, f_2p, f_2p+1, o_2p,
# o_2p+1, g_2p, g_2p+1] (torch gate order i,f,g,o = 0,1,2,3).
GATE_OF_Q = [0, 0, 1, 1, 3, 3, 2, 2]
PERM32 = [GATE_OF_Q[q] * 8 + (2 * p + (q % 2)) for p in range(4) for q in range(8)]


def split_multiwait(nc):
    """The walrus build in this environment rejects any instruction carrying
    more than one semaphore wait; hoist excess waits onto chained NOPs
    (sem-ge waits commute, so this preserves semantics)."""
    import bass_rust

    n_split = 0
    for f in nc.m.functions:
        for bb in f.blocks:
            new_insts = []
            changed = False
            for ins in bb.instructions:
                si = ins.sync_info
                if si is not None and si.on_wait and len(si.on_wait) > MAX_WAITS:
                    waits = list(si.on_wait)
                    extra, keep = waits[:-MAX_WAITS], waits[-MAX_WAITS:]
                    for j in range(0, len(extra), MAX_WAITS):
                        nop = bass_rust.InstNoOp(name=f"{ins.name}-wsplit{j}")
                        nop.engine = ins.engine
                        nop.sync_info = mybir.SyncInfo(
                            on_wait=extra[j : j + MAX_WAITS], on_update=[]
                        )
                        new_insts.append(nop)
                        n_split += 1
                    ins.sync_info = mybir.SyncInfo(
                        on_wait=keep, on_update=list(si.on_update)
                    )
                    changed = True
                new_insts.append(ins)
            if changed:
                bb.instructions = new_insts
    return n_split


W8_SCALE = 512.0
NCHUNK = 4  # outs AllGather chunks


def build_fused(T, v_loc=V // NCORES, timing=False, fp8=True):
    n_tok = B * T
    n_tt = n_tok // 128
    n_vc = v_loc // 500
    GSZ = 5 if fp8 else 10
    NG = n_tt // GSZ
    vh = v_loc // 4
    tc_len = T // NCHUNK
    assert n_tok % 128 == 0 and v_loc % 500 == 0 and T % 2 == 0
    assert n_tt % GSZ == 0 and T % NCHUNK == 0

    nc = bass.Bass(num_devices=NCORES)
    W8 = mybir.dt.float8e4 if fp8 else BF16
    inv_w8 = 1.0 / W8_SCALE if fp8 else 1.0
    whh_q = nc.declare_dram_parameter("whh_q", [2, 128, 32, 128], W8, isOutput=False)
    wih_q = nc.declare_dram_parameter("wih_q", [2, 128, 32, 128], BF16, isOutput=False)
    xT = nc.declare_dram_parameter("xT", [128, 8, 64], BF16, isOutput=False)
    h0T = nc.declare_dram_parameter("h0T", [128, 8, 64], W8, isOutput=False)
    c0T = nc.declare_dram_parameter("c0T", [128, 8, 64], F32, isOutput=False)
    biasT = nc.declare_dram_parameter("biasT", [128, 32], F32, isOutput=False)
    fcwT = nc.declare_dram_parameter("fcwT", [128, 16, v_loc], W8, isOutput=False)
    fcb = nc.declare_dram_parameter("fcb", [1, v_loc], BF16, isOutput=False)
    if timing:
        out = nc.dram_tensor("out", [n_tt, 128, v_loc], F32)
        chk = nc.declare_dram_parameter("chk", [128, 64], F32, isOutput=True)
    else:
        out = nc.declare_dram_parameter("out", [n_tt, 128, v_loc], F32, isOutput=True)

    ci_whh = nc.dram_tensor("ci_whh", [2, 128, 32, 128], W8)
    co_whh = nc.dram_tensor("co_whh", [8, 128, 32, 128], W8)
    ci_wih = nc.dram_tensor("ci_wih", [2, 128, 32, 128], BF16)
    co_wih = nc.dram_tensor("co_wih", [8, 128, 32, 128], BF16)
    outs_nat = nc.dram_tensor("outs_nat", [T, 128, 512], W8)
    # chunked pair-AllGather outputs: co[k] = [fwd tc_len | bwd tc_len]
    co = [nc.dram_tensor(f"co{k}", [2 * tc_len, 128, 512], W8) for k in range(NCHUNK)]
    cc_in = [nc.dram_tensor(f"cc_in{g}", [128, GSZ], F32) for g in range(NG)]
    cc_out = [nc.dram_tensor(f"cc_out{g}", [128, GSZ], F32) for g in range(NG)]

    ACT = mybir.ActivationFunctionType
    DIR_GROUPS = [[0, 1, 2, 3], [4, 5, 6, 7]]
    PAIR_GROUPS = [[0, 4], [1, 5], [2, 6], [3, 7]]
    ALL_GROUP = [list(range(NCORES))]

    with tile.TileContext(nc) as tc, ExitStack() as ctx:
        # ---- weight gather (params -> internal -> AllGather) ----
        nc.sync.dma_start(out=ci_whh[:], in_=whh_q[:])
        nc.sync.dma_start(out=ci_wih[:], in_=wih_q[:])
        nc.gpsimd.collective_compute(
            "AllGather", mybir.AluOpType.bypass, replica_groups=DIR_GROUPS,
            ins=[ci_whh[:]], outs=[co_whh[:]],
        )
        nc.gpsimd.collective_compute(
            "AllGather", mybir.AluOpType.bypass, replica_groups=DIR_GROUPS,
            ins=[ci_wih[:]], outs=[co_wih[:]],
        )

        # ---- scan phase (scoped pools so fc SBUF fits afterwards) ----
        with (
            tc.tile_pool(name="whh_pool", bufs=1) as whh_pool,
            tc.tile_pool(name="scan_work", bufs=2) as work,
            tc.tile_pool(name="scan_smalls", bufs=1) as smalls,
        ):
            bias_sb = smalls.tile([128, 32], F32)
            nc.sync.dma_start(out=bias_sb, in_=biasT[:])
            h_sb = smalls.tile([128, 8, 64], W8)
            nc.sync.dma_start(out=h_sb, in_=h0T[:])
            c_sb = smalls.tile([128, 8, 64], F32)
            nc.sync.dma_start(out=c_sb, in_=c0T[:])
            gx_sb = smalls.tile([128, 32, 64], F32)
            whh_sb = whh_pool.tile([128, 8, 32, 128], W8)
            for j in range(8):
                nc.sync.dma_start(out=whh_sb[:, j, :, :], in_=co_whh[j])

            with (
                tc.tile_pool(name="wih_pool", bufs=1) as wih_pool,
                tc.tile_pool(name="psum_gx", bufs=4, space="PSUM") as psum_gx,
            ):
                x_sb = smalls.tile([128, 8, 64], BF16)
                nc.sync.dma_start(out=x_sb, in_=xT[:])
                wih_sb = wih_pool.tile([128, 8, 32, 128], BF16)
                for j in range(8):
                    nc.sync.dma_start(out=wih_sb[:, j, :, :], in_=co_wih[j])
                for m in range(32):
                    pg = psum_gx.tile([128, 64], F32)
                    for j in range(8):
                        nc.tensor.matmul(
                            pg, wih_sb[:, j, m, :], x_sb[:, j, :],
                            start=(j == 0), stop=(j == 7),
                        )
                    nc.vector.tensor_scalar_add(
                        gx_sb[:, m, :], pg, bias_sb[:, m : m + 1]
                    )

            psum_scan_cm = tc.tile_pool(name="psum_scan", bufs=2, space="PSUM")
            psum_main = psum_scan_cm.__enter__()
            for t in range(T):
                pg = psum_main.tile([128, 32, 64], F32)
                for m in range(32):
                    if fp8:
                        for q in range(4):
                            nc.tensor.matmul(
                                pg[:, m, :],
                                whh_sb[:, 2 * q : 2 * q + 2, m, :],
                                h_sb[:, 2 * q : 2 * q + 2, :],
                                start=(q == 0), stop=(q == 3),
                                perf_mode=mybir.MatmulPerfMode.DoubleRow,
                                skip_group_check=True,
                            )
                    else:
                        for j in range(8):
                            nc.tensor.matmul(
                                pg[:, m, :], whh_sb[:, j, m, :], h_sb[:, j, :],
                                start=(j == 0), stop=(j == 7),
                            )
                gates = work.tile([128, 32, 64], F32, tag="gates")
                nl = work.tile([128, 32, 64], BF16, tag="nl")
                t1 = work.tile([128, 8, 64], BF16, tag="t1")
                t2 = work.tile([128, 8, 64], F32, tag="t2")
                tanhc = work.tile([128, 8, 64], BF16, tag="tanhc")
                for p in range(4):
                    ms = slice(8 * p, 8 * p + 8)
                    s2 = slice(2 * p, 2 * p + 2)
                    nc.vector.scalar_tensor_tensor(
                        gates[:, ms, :], pg[:, ms, :], inv_w8, gx_sb[:, ms, :],
                        op0=mybir.AluOpType.mult, op1=mybir.AluOpType.add,
                    )
                    nc.scalar.activation(
                        nl[:, 8 * p : 8 * p + 6, :],
                        gates[:, 8 * p : 8 * p + 6, :], ACT.Sigmoid,
                    )
                    nc.scalar.activation(
                        nl[:, 8 * p + 6 : 8 * p + 8, :],
                        gates[:, 8 * p + 6 : 8 * p + 8, :], ACT.Tanh,
                    )
                    nc.gpsimd.tensor_mul(
                        t1[:, s2, :], nl[:, 8 * p : 8 * p + 2, :],
                        nl[:, 8 * p + 6 : 8 * p + 8, :],
                    )
                    nc.gpsimd.tensor_mul(
                        t2[:, s2, :], nl[:, 8 * p + 2 : 8 * p + 4, :], c_sb[:, s2, :]
                    )
                    nc.vector.tensor_add(c_sb[:, s2, :], t1[:, s2, :], t2[:, s2, :])
                    nc.scalar.activation(tanhc[:, s2, :], c_sb[:, s2, :], ACT.Tanh)
                    nc.vector.tensor_mul(
                        h_sb[:, s2, :], nl[:, 8 * p + 4 : 8 * p + 6, :],
                        tanhc[:, s2, :],
                    )
                nc.sync.dma_start(
                    out=outs_nat[t], in_=h_sb.rearrange("p j b -> p (j b)")
                )
                if (t + 1) % tc_len == 0:
                    # pair AllGather of the finished chunk, overlapped with
                    # the remaining scan steps
                    k = t // tc_len
                    nc.gpsimd.collective_compute(
                        "AllGather", mybir.AluOpType.bypass,
                        replica_groups=PAIR_GROUPS,
                        ins=[outs_nat[k * tc_len : (k + 1) * tc_len]],
                        outs=[co[k][:]],
                    )

            psum_scan_cm.__exit__(None, None, None)

        # ---- fc phase: V-split, fp8 DoubleRow single pass, SBUF-resident
        # bf16 logits, grouped stats AllReduce with one-group-delayed epilogue
        fc_sing = ctx.enter_context(tc.tile_pool(name="fc_sing", bufs=1))
        w_sb = fc_sing.tile([128, 16, v_loc], W8)
        nc.sync.dma_start(out=w_sb, in_=fcwT[:])
        # bias is applied as a rank-1 extra contraction row: ones (stationary)
        # x fcb*W8_SCALE (moving); fcb shipped pre-scaled from host
        fcb_sb = fc_sing.tile([1, v_loc], BF16)
        nc.sync.dma_start(out=fcb_sb, in_=fcb[:])
        ones_sb = fc_sing.tile([1, 128], BF16)
        nc.vector.memset(ones_sb, 1.0)
        slab_pool = ctx.enter_context(tc.tile_pool(name="slabs", bufs=2))
        psum_fc = ctx.enter_context(tc.tile_pool(name="psum_fc", bufs=8, space="PSUM"))
        lgb_pool = ctx.enter_context(
            tc.tile_pool(name="fc_lgb", bufs=2 * GSZ + 1 if fp8 else 4)
        )
        ex_pool = ctx.enter_context(tc.tile_pool(name="fc_ex", bufs=1))
        epi_pool = ctx.enter_context(tc.tile_pool(name="fc_epi", bufs=2))
        spool = ctx.enter_context(tc.tile_pool(name="fc_stats", bufs=1))

        s_all = spool.tile([128, n_tt], F32)
        s_glob = spool.tile([128, n_tt], F32)
        nlz = spool.tile([128, n_tt], F32)
        ex = ex_pool.tile([128, v_loc], BF16)
        lgbs = {}
        inv_scale = 1.0 / W8_SCALE if fp8 else 1.0

        def pass_a(tt):
            o_t = slab_pool.tile([128, 16, 128], W8, tag="o_t")
            for d in range(2):
                sl = slab_pool.tile([128, 2, 512], W8, tag=f"sl{d}")
                for t2 in range(2):
                    tg = 2 * tt + t2
                    nc.sync.dma_start(
                        out=sl[:, t2, :],
                        in_=co[tg // tc_len][d * tc_len + tg % tc_len],
                    )
                nc.vector.tensor_copy(
                    o_t[:, d * 8 : d * 8 + 8, :].rearrange(
                        "p j (t b) -> p j t b", t=2
                    ),
                    sl.rearrange("p t (j b) -> p j t b", j=8),
                )
            lgb = lgb_pool.tile([128, v_loc], BF16, tag="lgb")
            lgbs[tt] = lgb
            for c in range(n_vc):
                vs = slice(c * 500, (c + 1) * 500)
                ps = psum_fc.tile([128, 500], F32)
                if fp8:
                    for k2 in range(8):
                        nc.tensor.matmul(
                            ps, o_t[:, 2 * k2 : 2 * k2 + 2, :],
                            w_sb[:, 2 * k2 : 2 * k2 + 2, vs],
                            start=(k2 == 0), stop=False,
                            perf_mode=mybir.MatmulPerfMode.DoubleRow,
                            skip_group_check=True,
                        )
                else:
                    for kt in range(16):
                        nc.tensor.matmul(
                            ps, o_t[:, kt, :], w_sb[:, kt, vs],
                            start=(kt == 0), stop=False,
                            skip_group_check=True,
                        )
                nc.tensor.matmul(
                    ps, ones_sb, fcb_sb[:, vs],
                    start=False, stop=True, skip_group_check=True,
                )
                nc.vector.tensor_scalar_mul(lgb[:, vs], ps, inv_scale)
            nc.scalar.activation(
                ex, lgb, ACT.Exp, accum_out=s_all[:, tt : tt + 1]
            )

        def stats(g):
            gs = slice(g * GSZ, (g + 1) * GSZ)
            nc.gpsimd.dma_start(out=cc_in[g][:], in_=s_all[:, gs])
            nc.gpsimd.collective_compute(
                "AllReduce", mybir.AluOpType.add, replica_groups=ALL_GROUP,
                ins=[cc_in[g][:]], outs=[cc_out[g][:]],
            )
            nc.gpsimd.dma_start(out=s_glob[:, gs], in_=cc_out[g][:])
            nc.scalar.activation(nlz[:, gs], s_glob[:, gs], ACT.Ln)
            nc.vector.tensor_scalar_mul(nlz[:, gs], nlz[:, gs], -1.0)

        def epilogue(tt):
            lgb = lgbs.pop(tt)
            for h2 in range(4):
                vs = slice(h2 * vh, (h2 + 1) * vh)
                outf = epi_pool.tile([128, vh], F32, tag="outf")
                nc.scalar.activation(
                    outf, lgb[:, vs], ACT.Identity, bias=nlz[:, tt : tt + 1]
                )
                nc.sync.dma_start(out=out[tt][:, vs], in_=outf)

        for g in range(NG):
            for tt in range(g * GSZ, (g + 1) * GSZ):
                pass_a(tt)
            stats(g)
            if g >= 1:
                for tt in range((g - 1) * GSZ, g * GSZ):
                    epilogue(tt)
        for tt in range((NG - 1) * GSZ, NG * GSZ):
            epilogue(tt)

        if timing:
            chk_sb = spool.tile([128, 64], F32)
            nc.vector.tensor_copy(chk_sb[:, :n_tt], nlz)
            nc.sync.dma_start(out=chk[:, :n_tt], in_=chk_sb[:, :n_tt])

    split_multiwait(nc)
    return nc


def prep_fused_inputs(x, h0, c0, W_ih, W_hh, b_ih, b_hh, fc_W, fc_b, T):
    """Per-core in_maps. Core c: direction d = c//4, weight quarter qc = c%4,
    vocab slice c. Gate rows are permuted to pair-major chunk order PERM32."""
    v_loc = V // NCORES
    maps = []
    per_dir = {}
    for d in (0, 1):
        whh_p = W_hh[d].reshape(32, 128, H)[PERM32]
        wih_p = W_ih[d].reshape(32, 128, H)[PERM32]
        bias_p = (b_ih[d] + b_hh[d]).reshape(32, 128)[PERM32]
        whh_full = whh_p.reshape(32, 128, 8, 128).transpose(3, 2, 0, 1)  # [p,j,m,q]
        wih_full = wih_p.reshape(32, 128, 8, 128).transpose(3, 2, 0, 1)
        per_dir[d] = {
            "whh": whh_full,
            "wih": wih_full,
            "h0T": np.ascontiguousarray(
                h0[d].reshape(64, 8, 128).transpose(2, 1, 0)
            ).astype(NP_W8),
            "c0T": np.ascontiguousarray(
                c0[d].reshape(64, 8, 128).transpose(2, 1, 0)
            ).astype(np.float32),
            "biasT": np.ascontiguousarray(bias_p.T).astype(np.float32),
        }
    xT = np.ascontiguousarray(x.reshape(64, 8, 128).transpose(2, 1, 0)).astype(
        NP_BF16
    )
    for c in range(NCORES):
        d, qc = c // 4, c % 4
        pd = per_dir[d]
        whh_q = np.ascontiguousarray(
            pd["whh"][:, 2 * qc : 2 * qc + 2].transpose(1, 0, 2, 3) * W8_SCALE
        ).astype(NP_W8)
        wih_q = np.ascontiguousarray(
            pd["wih"][:, 2 * qc : 2 * qc + 2].transpose(1, 0, 2, 3)
        ).astype(NP_BF16)
        wv = fc_W[c * v_loc : (c + 1) * v_loc]
        fcwT = np.ascontiguousarray(
            wv.reshape(v_loc, 16, 128).transpose(2, 1, 0) * W8_SCALE
        ).astype(NP_W8)
        maps.append(
            {
                "whh_q": whh_q,
                "wih_q": wih_q,
                "xT": xT,
                "h0T": pd["h0T"],
                "c0T": pd["c0T"],
                "biasT": pd["biasT"],
                "fcwT": fcwT,
                "fcb": np.ascontiguousarray(
                    fc_b[c * v_loc : (c + 1) * v_loc].reshape(1, v_loc) * W8_SCALE
                ).astype(NP_BF16),
            }
        )
    return maps


def assemble_output(results, T):
    """results[c]["out"] is [n_tt, 128, v_loc], token = t*64 + b."""
    v_loc = V // NCORES
    full = np.concatenate(
        [results[c]["out"].reshape(B * T, v_loc) for c in range(NCORES)], axis=1
    )  # [t*64+b, V]
    return np.ascontiguousarray(
        full.reshape(T, B, V).transpose(1, 0, 2)
    )


_build_cache = {}


def kernel(x, h0, c0, W_ih, W_hh, b_ih, b_hh, fc_W, fc_b, max_len):
    T = int(max_len)
    x = np.asarray(x, np.float32)
    h0 = np.asarray(h0, np.float32)
    c0 = np.asarray(c0, np.float32)
    W_ih = np.asarray(W_ih, np.float32)
    W_hh = np.asarray(W_hh, np.float32)
    b_ih = np.asarray(b_ih, np.float32)
    b_hh = np.asarray(b_hh, np.float32)
    fc_W = np.asarray(fc_W, np.float32)
    fc_b = np.asarray(fc_b, np.float32)

    if T not in _build_cache:
        _build_cache[T] = build_fused(T)
    nc = _build_cache[T]
    maps = prep_fused_inputs(
        x, h0, c0, W_ih, W_hh, b_ih, b_hh, fc_W, fc_b, T
    )
    res = run_bass_kernel_spmd(nc, maps, core_ids=list(range(NCORES)))
    return assemble_output([res.results[c] for c in range(NCORES)], T)


# revision 21
# speedup vs baseline: 1.1710x; 1.1710x over previous
"""Self-contained Trainium2 Bass kernel for the bidirectional-LSTM decoder
(nn_Decoder): 2-direction LSTM scan over T=100 steps with a fixed input,
followed by a 32000-way vocab projection and log_softmax, on 8 NeuronCores.

Distribution (single fused SPMD launch):
  - Scan: direction-split. Cores 0-3 compute the forward LSTM for all 64
    batch rows, cores 4-7 the backward one. Each core receives a quarter of
    its direction's weights; full matrices are reassembled on-device with
    AllGather so host->device staging stays small.
  - Transpose-free scan layout: gates^T[4H, B] accumulated as
    sum_j Whh^T-tile[j, m].T @ h^T-chunk[j], state kept as [128, j, b].
    Gate rows are PERMUTED host-side to pair-major order
    (i,i,f,f,o,o,g,g per slice pair) so the per-step nonlinearity +
    cell update runs at slice-pair granularity pipelined against the
    matmuls of later gate chunks (sub-tile dependency tracking), instead
    of one serial chain after the full 4H x B gate tile.
  - LSTM outputs stay on-device: pair-AllGather gives every core both
    directions.
  - fc + log_softmax: vocab-split (4000 rows per core, fc_W resident in
    SBUF, bf16). SINGLE matmul pass: logits (+bias) are written once as
    bf16 both to SBUF stats (exp + accumulate -> softmax partial sums)
    and to internal DRAM. Partial exp-sums are AllReduced in 5 groups of
    10 token-tiles so the normalizer exchange and the epilogue overlap
    the remaining matmul work; the epilogue re-reads the bf16 logits,
    adds -logZ via an Identity activation with per-partition bias, and
    writes the final fp32 output. This replaces the baseline's full
    second recompute pass (~1.3 ms of PE time) with ~100 MB of
    overlapped DMA traffic.
  - All matmuls run in bf16 (fp32 accumulation in PSUM); cell state c
    stays fp32.
"""

import sys

if "/opt/trn_rl_repo" not in sys.path:
    sys.path.insert(0, "/opt/trn_rl_repo")

from contextlib import ExitStack

import numpy as np

import concourse.bass as bass
import concourse.tile as tile
from concourse import mybir
from concourse.bass_utils import run_bass_kernel_spmd

F32 = mybir.dt.float32
BF16 = mybir.dt.bfloat16
NP_BF16 = mybir.dt.np(BF16)
NP_W8 = mybir.dt.np(mybir.dt.float8e4)
B = 64
H = 1024
V = 32000
NCORES = 8

MAX_WAITS = 1

# Pair-major gate-chunk permutation: new chunk n (of 32) -> original chunk
# gate*8 + slice, with per-pair order [i_2p, i_2p+1, f_2p, f_2p+1, o_2p,
# o_2p+1, g_2p, g_2p+1] (torch gate order i,f,g,o = 0,1,2,3).
GATE_OF_Q = [0, 0, 1, 1, 3, 3, 2, 2]
PERM32 = [GATE_OF_Q[q] * 8 + (2 * p + (q % 2)) for p in range(4) for q in range(8)]


def split_multiwait(nc):
    """The walrus build in this environment rejects any instruction carrying
    more than one semaphore wait; hoist excess waits onto chained NOPs
    (sem-ge waits commute, so this preserves semantics)."""
    import bass_rust

    n_split = 0
    for f in nc.m.functions:
        for bb in f.blocks:
            new_insts = []
            changed = False
            for ins in bb.instructions:
                si = ins.sync_info
                if si is not None and si.on_wait and len(si.on_wait) > MAX_WAITS:
                    waits = list(si.on_wait)
                    extra, keep = waits[:-MAX_WAITS], waits[-MAX_WAITS:]
                    for j in range(0, len(extra), MAX_WAITS):
                        nop = bass_rust.InstNoOp(name=f"{ins.name}-wsplit{j}")
                        nop.engine = ins.engine
                        nop.sync_info = mybir.SyncInfo(
                            on_wait=extra[j : j + MAX_WAITS], on_update=[]
                        )
                        new_insts.append(nop)
                        n_split += 1
                    ins.sync_info = mybir.SyncInfo(
                        on_wait=keep, on_update=list(si.on_update)
                    )
                    changed = True
                new_insts.append(ins)
            if changed:
                bb.instructions = new_insts
    return n_split


W8_SCALE = 512.0
NCHUNK = 4  # outs AllGather chunks


def build_fused(T, v_loc=V // NCORES, timing=False, fp8=True):
    n_tok = B * T
    n_tt = n_tok // 128
    n_vc = v_loc // 500
    GSZ = 5 if fp8 else 10
    NG = n_tt // GSZ
    vh = v_loc // 4
    tc_len = T // NCHUNK
    assert n_tok % 128 == 0 and v_loc % 500 == 0 and T % 2 == 0
    assert n_tt % GSZ == 0 and T % NCHUNK == 0

    nc = bass.Bass(num_devices=NCORES)
    W8 = mybir.dt.float8e4 if fp8 else BF16
    inv_w8 = 1.0 / W8_SCALE if fp8 else 1.0
    whh_q = nc.declare_dram_parameter("whh_q", [2, 128, 32, 128], BF16, isOutput=False)
    wih_q = nc.declare_dram_parameter("wih_q", [2, 128, 32, 128], BF16, isOutput=False)
    xT = nc.declare_dram_parameter("xT", [128, 8, 64], BF16, isOutput=False)
    h0T = nc.declare_dram_parameter("h0T", [128, 8, 64], BF16, isOutput=False)
    c0T = nc.declare_dram_parameter("c0T", [128, 8, 64], F32, isOutput=False)
    biasT = nc.declare_dram_parameter("biasT", [128, 32], F32, isOutput=False)
    fcwT = nc.declare_dram_parameter("fcwT", [128, 16, v_loc], W8, isOutput=False)
    fcb = nc.declare_dram_parameter("fcb", [1, v_loc], BF16, isOutput=False)
    if timing:
        out = nc.dram_tensor("out", [n_tt, 128, v_loc], F32)
        chk = nc.declare_dram_parameter("chk", [128, 64], F32, isOutput=True)
    else:
        out = nc.declare_dram_parameter("out", [n_tt, 128, v_loc], F32, isOutput=True)

    ci_whh = nc.dram_tensor("ci_whh", [2, 128, 32, 128], BF16)
    co_whh = nc.dram_tensor("co_whh", [8, 128, 32, 128], BF16)
    ci_wih = nc.dram_tensor("ci_wih", [2, 128, 32, 128], BF16)
    co_wih = nc.dram_tensor("co_wih", [8, 128, 32, 128], BF16)
    outs_nat = nc.dram_tensor("outs_nat", [T, 128, 512], BF16)
    # chunked pair-AllGather outputs: co[k] = [fwd tc_len | bwd tc_len]
    co = [nc.dram_tensor(f"co{k}", [2 * tc_len, 128, 512], BF16) for k in range(NCHUNK)]
    cc_in = [nc.dram_tensor(f"cc_in{g}", [128, GSZ], F32) for g in range(NG)]
    cc_out = [nc.dram_tensor(f"cc_out{g}", [128, GSZ], F32) for g in range(NG)]

    ACT = mybir.ActivationFunctionType
    DIR_GROUPS = [[0, 1, 2, 3], [4, 5, 6, 7]]
    PAIR_GROUPS = [[0, 4], [1, 5], [2, 6], [3, 7]]
    ALL_GROUP = [list(range(NCORES))]

    with tile.TileContext(nc) as tc, ExitStack() as ctx:
        # ---- weight gather (params -> internal -> AllGather) ----
        nc.sync.dma_start(out=ci_whh[:], in_=whh_q[:])
        nc.sync.dma_start(out=ci_wih[:], in_=wih_q[:])
        nc.gpsimd.collective_compute(
            "AllGather", mybir.AluOpType.bypass, replica_groups=DIR_GROUPS,
            ins=[ci_whh[:]], outs=[co_whh[:]],
        )
        nc.gpsimd.collective_compute(
            "AllGather", mybir.AluOpType.bypass, replica_groups=DIR_GROUPS,
            ins=[ci_wih[:]], outs=[co_wih[:]],
        )

        # ---- scan phase (scoped pools so fc SBUF fits afterwards) ----
        with (
            tc.tile_pool(name="whh_pool", bufs=1) as whh_pool,
            tc.tile_pool(name="scan_work", bufs=2) as work,
            tc.tile_pool(name="scan_smalls", bufs=1) as smalls,
        ):
            bias_sb = smalls.tile([128, 32], F32)
            nc.sync.dma_start(out=bias_sb, in_=biasT[:])
            h_sb = smalls.tile([128, 8, 64], BF16)
            nc.sync.dma_start(out=h_sb, in_=h0T[:])
            c_sb = smalls.tile([128, 8, 64], F32)
            nc.sync.dma_start(out=c_sb, in_=c0T[:])
            gx_sb = smalls.tile([128, 32, 64], F32)
            whh_sb = whh_pool.tile([128, 8, 32, 128], BF16)
            for j in range(8):
                nc.sync.dma_start(out=whh_sb[:, j, :, :], in_=co_whh[j])

            with (
                tc.tile_pool(name="wih_pool", bufs=1) as wih_pool,
                tc.tile_pool(name="psum_gx", bufs=4, space="PSUM") as psum_gx,
            ):
                x_sb = smalls.tile([128, 8, 64], BF16)
                nc.sync.dma_start(out=x_sb, in_=xT[:])
                wih_sb = wih_pool.tile([128, 8, 32, 128], BF16)
                for j in range(8):
                    nc.sync.dma_start(out=wih_sb[:, j, :, :], in_=co_wih[j])
                for m in range(32):
                    pg = psum_gx.tile([128, 64], F32)
                    for j in range(8):
                        nc.tensor.matmul(
                            pg, wih_sb[:, j, m, :], x_sb[:, j, :],
                            start=(j == 0), stop=(j == 7),
                        )
                    nc.vector.tensor_scalar_add(
                        gx_sb[:, m, :], pg, bias_sb[:, m : m + 1]
                    )

            psum_scan_cm = tc.tile_pool(name="psum_scan", bufs=2, space="PSUM")
            psum_main = psum_scan_cm.__enter__()
            for t in range(T):
                pg = psum_main.tile([128, 32, 64], F32)
                for m in range(32):
                    for j in range(8):
                        nc.tensor.matmul(
                            pg[:, m, :], whh_sb[:, j, m, :], h_sb[:, j, :],
                            start=(j == 0), stop=(j == 7),
                        )
                gates = work.tile([128, 32, 64], F32, tag="gates")
                nl = work.tile([128, 32, 64], BF16, tag="nl")
                t1 = work.tile([128, 8, 64], BF16, tag="t1")
                t2 = work.tile([128, 8, 64], F32, tag="t2")
                tanhc = work.tile([128, 8, 64], BF16, tag="tanhc")
                for p in range(4):
                    ms = slice(8 * p, 8 * p + 8)
                    s2 = slice(2 * p, 2 * p + 2)
                    nc.vector.tensor_add(gates[:, ms, :], pg[:, ms, :], gx_sb[:, ms, :])
                    nc.scalar.activation(
                        nl[:, 8 * p : 8 * p + 6, :],
                        gates[:, 8 * p : 8 * p + 6, :], ACT.Sigmoid,
                    )
                    nc.scalar.activation(
                        nl[:, 8 * p + 6 : 8 * p + 8, :],
                        gates[:, 8 * p + 6 : 8 * p + 8, :], ACT.Tanh,
                    )
                    nc.vector.tensor_mul(
                        t1[:, s2, :], nl[:, 8 * p : 8 * p + 2, :],
                        nl[:, 8 * p + 6 : 8 * p + 8, :],
                    )
                    nc.vector.tensor_mul(
                        t2[:, s2, :], nl[:, 8 * p + 2 : 8 * p + 4, :], c_sb[:, s2, :]
                    )
                    nc.vector.tensor_add(c_sb[:, s2, :], t1[:, s2, :], t2[:, s2, :])
                    nc.scalar.activation(tanhc[:, s2, :], c_sb[:, s2, :], ACT.Tanh)
                    nc.vector.tensor_mul(
                        h_sb[:, s2, :], nl[:, 8 * p + 4 : 8 * p + 6, :],
                        tanhc[:, s2, :],
                    )
                nc.sync.dma_start(
                    out=outs_nat[t], in_=h_sb.rearrange("p j b -> p (j b)")
                )
                if (t + 1) % tc_len == 0:
                    # pair AllGather of the finished chunk, overlapped with
                    # the remaining scan steps
                    k = t // tc_len
                    nc.gpsimd.collective_compute(
                        "AllGather", mybir.AluOpType.bypass,
                        replica_groups=PAIR_GROUPS,
                        ins=[outs_nat[k * tc_len : (k + 1) * tc_len]],
                        outs=[co[k][:]],
                    )

            psum_scan_cm.__exit__(None, None, None)

        # ---- fc phase: V-split, fp8 DoubleRow single pass, SBUF-resident
        # bf16 logits, grouped stats AllReduce with one-group-delayed epilogue
        fc_sing = ctx.enter_context(tc.tile_pool(name="fc_sing", bufs=1))
        w_sb = fc_sing.tile([128, 16, v_loc], W8)
        nc.sync.dma_start(out=w_sb, in_=fcwT[:])
        # bias is applied as a rank-1 extra contraction row: ones (stationary)
        # x fcb*W8_SCALE (moving); fcb shipped pre-scaled from host
        fcb_sb = fc_sing.tile([1, v_loc], BF16)
        nc.sync.dma_start(out=fcb_sb, in_=fcb[:])
        ones_sb = fc_sing.tile([1, 128], BF16)
        nc.vector.memset(ones_sb, 1.0)
        slab_pool = ctx.enter_context(tc.tile_pool(name="slabs", bufs=2))
        psum_fc = ctx.enter_context(tc.tile_pool(name="psum_fc", bufs=8, space="PSUM"))
        lgb_pool = ctx.enter_context(
            tc.tile_pool(name="fc_lgb", bufs=2 * GSZ + 1 if fp8 else 4)
        )
        ex_pool = ctx.enter_context(tc.tile_pool(name="fc_ex", bufs=1))
        epi_pool = ctx.enter_context(tc.tile_pool(name="fc_epi", bufs=2))
        spool = ctx.enter_context(tc.tile_pool(name="fc_stats", bufs=1))

        s_all = spool.tile([128, n_tt], F32)
        s_glob = spool.tile([128, n_tt], F32)
        nlz = spool.tile([128, n_tt], F32)
        ex = ex_pool.tile([128, v_loc], BF16)
        lgbs = {}
        inv_scale = 1.0 / W8_SCALE if fp8 else 1.0

        def pass_a(tt):
            o_t = slab_pool.tile([128, 16, 128], W8, tag="o_t")
            for d in range(2):
                sl = slab_pool.tile([128, 2, 512], BF16, tag=f"sl{d}")
                for t2 in range(2):
                    tg = 2 * tt + t2
                    nc.sync.dma_start(
                        out=sl[:, t2, :],
                        in_=co[tg // tc_len][d * tc_len + tg % tc_len],
                    )
                nc.vector.tensor_copy(
                    o_t[:, d * 8 : d * 8 + 8, :].rearrange(
                        "p j (t b) -> p j t b", t=2
                    ),
                    sl.rearrange("p t (j b) -> p j t b", j=8),
                )
            lgb = lgb_pool.tile([128, v_loc], BF16, tag="lgb")
            lgbs[tt] = lgb
            for c in range(n_vc):
                vs = slice(c * 500, (c + 1) * 500)
                ps = psum_fc.tile([128, 500], F32)
                if fp8:
                    for k2 in range(8):
                        nc.tensor.matmul(
                            ps, o_t[:, 2 * k2 : 2 * k2 + 2, :],
                            w_sb[:, 2 * k2 : 2 * k2 + 2, vs],
                            start=(k2 == 0), stop=False,
                            perf_mode=mybir.MatmulPerfMode.DoubleRow,
                            skip_group_check=True,
                        )
                else:
                    for kt in range(16):
                        nc.tensor.matmul(
                            ps, o_t[:, kt, :], w_sb[:, kt, vs],
                            start=(kt == 0), stop=False,
                            skip_group_check=True,
                        )
                nc.tensor.matmul(
                    ps, ones_sb, fcb_sb[:, vs],
                    start=False, stop=True, skip_group_check=True,
                )
                nc.vector.tensor_scalar_mul(lgb[:, vs], ps, inv_scale)
            nc.scalar.activation(
                ex, lgb, ACT.Exp, accum_out=s_all[:, tt : tt + 1]
            )

        def stats(g):
            gs = slice(g * GSZ, (g + 1) * GSZ)
            nc.gpsimd.dma_start(out=cc_in[g][:], in_=s_all[:, gs])
            nc.gpsimd.collective_compute(
                "AllReduce", mybir.AluOpType.add, replica_groups=ALL_GROUP,
                ins=[cc_in[g][:]], outs=[cc_out[g][:]],
            )
            nc.gpsimd.dma_start(out=s_glob[:, gs], in_=cc_out[g][:])
            nc.scalar.activation(nlz[:, gs], s_glob[:, gs], ACT.Ln)
            nc.vector.tensor_scalar_mul(nlz[:, gs], nlz[:, gs], -1.0)

        def epilogue(tt):
            lgb = lgbs.pop(tt)
            for h2 in range(4):
                vs = slice(h2 * vh, (h2 + 1) * vh)
                outf = epi_pool.tile([128, vh], F32, tag="outf")
                nc.scalar.activation(
                    outf, lgb[:, vs], ACT.Identity, bias=nlz[:, tt : tt + 1]
                )
                nc.sync.dma_start(out=out[tt][:, vs], in_=outf)

        for g in range(NG):
            for tt in range(g * GSZ, (g + 1) * GSZ):
                pass_a(tt)
            stats(g)
            if g >= 1:
                for tt in range((g - 1) * GSZ, g * GSZ):
                    epilogue(tt)
        for tt in range((NG - 1) * GSZ, NG * GSZ):
            epilogue(tt)

        if timing:
            chk_sb = spool.tile([128, 64], F32)
            nc.vector.tensor_copy(chk_sb[:, :n_tt], nlz)
            nc.sync.dma_start(out=chk[:, :n_tt], in_=chk_sb[:, :n_tt])

    split_multiwait(nc)
    return nc


def prep_fused_inputs(x, h0, c0, W_ih, W_hh, b_ih, b_hh, fc_W, fc_b, T):
    """Per-core in_maps. Core c: direction d = c//4, weight quarter qc = c%4,
    vocab slice c. Gate rows are permuted to pair-major chunk order PERM32."""
    v_loc = V // NCORES
    maps = []
    per_dir = {}
    for d in (0, 1):
        whh_p = W_hh[d].reshape(32, 128, H)[PERM32]
        wih_p = W_ih[d].reshape(32, 128, H)[PERM32]
        bias_p = (b_ih[d] + b_hh[d]).reshape(32, 128)[PERM32]
        whh_full = whh_p.reshape(32, 128, 8, 128).transpose(3, 2, 0, 1)  # [p,j,m,q]
        wih_full = wih_p.reshape(32, 128, 8, 128).transpose(3, 2, 0, 1)
        per_dir[d] = {
            "whh": whh_full,
            "wih": wih_full,
            "h0T": np.ascontiguousarray(
                h0[d].reshape(64, 8, 128).transpose(2, 1, 0)
            ).astype(NP_BF16),
            "c0T": np.ascontiguousarray(
                c0[d].reshape(64, 8, 128).transpose(2, 1, 0)
            ).astype(np.float32),
            "biasT": np.ascontiguousarray(bias_p.T).astype(np.float32),
        }
    xT = np.ascontiguousarray(x.reshape(64, 8, 128).transpose(2, 1, 0)).astype(
        NP_BF16
    )
    for c in range(NCORES):
        d, qc = c // 4, c % 4
        pd = per_dir[d]
        whh_q = np.ascontiguousarray(
            pd["whh"][:, 2 * qc : 2 * qc + 2].transpose(1, 0, 2, 3)
        ).astype(NP_BF16)
        wih_q = np.ascontiguousarray(
            pd["wih"][:, 2 * qc : 2 * qc + 2].transpose(1, 0, 2, 3)
        ).astype(NP_BF16)
        wv = fc_W[c * v_loc : (c + 1) * v_loc]
        fcwT = np.ascontiguousarray(
            wv.reshape(v_loc, 16, 128).transpose(2, 1, 0) * W8_SCALE
        ).astype(NP_W8)
        maps.append(
            {
                "whh_q": whh_q,
                "wih_q": wih_q,
                "xT": xT,
                "h0T": pd["h0T"],
                "c0T": pd["c0T"],
                "biasT": pd["biasT"],
                "fcwT": fcwT,
                "fcb": np.ascontiguousarray(
                    fc_b[c * v_loc : (c + 1) * v_loc].reshape(1, v_loc) * W8_SCALE
                ).astype(NP_BF16),
            }
        )
    return maps


def assemble_output(results, T):
    """results[c]["out"] is [n_tt, 128, v_loc], token = t*64 + b."""
    v_loc = V // NCORES
    full = np.concatenate(
        [results[c]["out"].reshape(B * T, v_loc) for c in range(NCORES)], axis=1
    )  # [t*64+b, V]
    return np.ascontiguousarray(
        full.reshape(T, B, V).transpose(1, 0, 2)
    )


_build_cache = {}


def kernel(x, h0, c0, W_ih, W_hh, b_ih, b_hh, fc_W, fc_b, max_len):
    T = int(max_len)
    x = np.asarray(x, np.float32)
    h0 = np.asarray(h0, np.float32)
    c0 = np.asarray(c0, np.float32)
    W_ih = np.asarray(W_ih, np.float32)
    W_hh = np.asarray(W_hh, np.float32)
    b_ih = np.asarray(b_ih, np.float32)
    b_hh = np.asarray(b_hh, np.float32)
    fc_W = np.asarray(fc_W, np.float32)
    fc_b = np.asarray(fc_b, np.float32)

    if T not in _build_cache:
        _build_cache[T] = build_fused(T)
    nc = _build_cache[T]
    maps = prep_fused_inputs(
        x, h0, c0, W_ih, W_hh, b_ih, b_hh, fc_W, fc_b, T
    )
    res = run_bass_kernel_spmd(nc, maps, core_ids=list(range(NCORES)))
    return assemble_output([res.results[c] for c in range(NCORES)], T)
